# revision 26
# baseline (speedup 1.0000x reference)
"""CrossAttention Trainium2 kernel.

Full inputs in, full output out. Data-parallel over batch: core b computes
batch item b of 8.

Per-core math (layouts chosen so the PE contraction dim is always the
partition dim, no on-chip transposes):
  QT[d, q] = (Wq*scale @ q_b^T)      lhsT=wq chunks, rhs=q_b^T    (bf16)
  KT[d, k] = (Wk @ kv_b^T)                                        (bf16)
  V[k, d]  = (kv_b @ Wv^T)           + ones column -> rowsum row 64
  S^T[k, q] = K Q^T per head         (64-contraction)
  P^T = exp(S^T - ln256) * exp_posT  (exp_pos host-precomputed fp16)
  O^T[d, q] rows 0..63 (+rowsum row 64): psum-accumulated over k chunks
  XT = O^T[0:64] * (1/rowsum)        rowsum row copied to partition 0
                                     (custom DVE recip needs base-0 input;
                                     64-wide accesses must be 32-aligned)
  out[q, e] = XT^T @ WprojT + bias

Schedule: attention S-matmuls run 3 k-chunks ahead of the O-matmuls so the
exp->mul chain (scalar + pool/vector engines) never stalls the PE; epos
multiplies alternate between the Pool and Vector engines.
"""

import numpy as np

B, L, DIM, H, HD = 8, 1024, 768, 12, 64
NCORES = 8
CP = DIM // 128  # 6 chunks of the contraction/feature dim
KC = L // 128    # 8 k-chunks
SCALE = HD ** -0.5
LN_OFF = float(np.log(256.0))

_CACHE = {}


def _build():
    import concourse.bass as bass
    import concourse.mybir as mybir
    import concourse.tile as tile
    from concourse import bacc

    f32 = mybir.dt.float32
    bf16 = mybir.dt.bfloat16
    f16 = mybir.dt.float16
    f8 = mybir.dt.float8e4
    DR = mybir.MatmulPerfMode.DoubleRow
    AF = mybir.ActivationFunctionType

    nc = bacc.Bacc("TRN2", target_bir_lowering=False, debug=False)

    qT = nc.dram_tensor("qT", [DIM, L], bf16, kind="ExternalInput")
    kvT = nc.dram_tensor("kvT", [DIM, L], bf16, kind="ExternalInput")
    wq = nc.dram_tensor("wq", [DIM, DIM], bf16, kind="ExternalInput")  # [c, d]
    wk = nc.dram_tensor("wk", [DIM, DIM], bf16, kind="ExternalInput")  # [c, d]
    wv = nc.dram_tensor("wv", [DIM, DIM], bf16, kind="ExternalInput")  # [c, d]
    wp = nc.dram_tensor("wp", [DIM, DIM], bf16, kind="ExternalInput")  # [d, e]
    bias = nc.dram_tensor("bias", [128, DIM], f32, kind="ExternalInput")
    epos = nc.dram_tensor("epos", [H, L, L], f16, kind="ExternalInput")  # [h,k,q]
    out = nc.dram_tensor("out", [L, DIM], f32, kind="ExternalOutput")
    rscr = nc.dram_tensor("rs_scratch", [H, L], f32)

    with tile.TileContext(nc) as tc:
        with tc.tile_pool(name="persist", bufs=1) as persist:
            QT = persist.tile([128, CP, L], bf16)   # pair p: heads 2p, 2p+1
            KT = persist.tile([128, CP, L], bf16)
            Vt = [
                persist.tile([128, H, HD + 1], f16, name=f"Vt{k}", tag=f"V{k}")
                for k in range(KC)
            ]
            wp_sb = persist.tile([128, CP, DIM], bf16)
            bias_bc = persist.tile([128, DIM], f32)
            XT = persist.tile([128, CP, L], bf16)
            expb = persist.tile([128, 1], f32)
            nc.vector.memset(expb[:], -LN_OFF)

            # ---------------- phase 1: projections ----------------
            with (
                tc.tile_pool(name="ph1", bufs=1) as ph1,
                tc.tile_pool(name="psA", bufs=2, space="PSUM") as psA,
            ):
                q_sb = ph1.tile([128, CP, L], bf16)
                kv_sb = ph1.tile([128, CP, L], bf16)
                wq_sb = ph1.tile([128, CP, DIM], bf16)
                wk_sb = ph1.tile([128, CP, DIM], bf16)
                wv_sb = ph1.tile([128, CP, DIM], bf16)
                # issue DMAs in need-order: Q-proj inputs first
                for c in range(CP):
                    nc.sync.dma_start(wq_sb[:, c, :], wq[c * 128:(c + 1) * 128, :])
                    nc.sync.dma_start(q_sb[:, c, :], qT[c * 128:(c + 1) * 128, :])
                for c in range(CP):
                    nc.sync.dma_start(wk_sb[:, c, :], wk[c * 128:(c + 1) * 128, :])
                    nc.sync.dma_start(kv_sb[:, c, :], kvT[c * 128:(c + 1) * 128, :])
                for c in range(CP):
                    nc.sync.dma_start(wv_sb[:, c, :], wv[c * 128:(c + 1) * 128, :])
                for d in range(CP):
                    nc.sync.dma_start(wp_sb[:, d, :], wp[d * 128:(d + 1) * 128, :])
                nc.sync.dma_start(bias_bc[:], bias[:])

                for w_sb, x_sb, dst in ((wq_sb, q_sb, QT), (wk_sb, kv_sb, KT)):
                    for p in range(CP):
                        ps = psA.tile([128, L], f32, tag="proj")
                        for c in range(CP):
                            for hf in range(2):
                                nc.tensor.matmul(
                                    ps[:, hf * 512:(hf + 1) * 512],
                                    w_sb[:, c, p * 128:(p + 1) * 128],
                                    x_sb[:, c, hf * 512:(hf + 1) * 512],
                                    start=(c == 0),
                                    stop=(c == CP - 1),
                                )
                        nc.vector.tensor_copy(dst[:, p, :], ps[:])

                for k in range(KC):
                    ps = psA.tile([128, DIM], f32, tag="proj")
                    for c in range(CP):
                        for lo, sz in ((0, 512), (512, 256)):
                            nc.tensor.matmul(
                                ps[:, lo:lo + sz],
                                kv_sb[:, c, k * 128:(k + 1) * 128],
                                wv_sb[:, c, lo:lo + sz],
                                start=(c == 0),
                                stop=(c == CP - 1),
                            )
                    nc.vector.memset(Vt[k][:, :, HD:HD + 1], 1.0)
                    nc.vector.tensor_copy(
                        Vt[k][:, :, 0:HD],
                        ps[:].rearrange("p (h d) -> p h d", d=HD),
                    )

            # ---------------- phase 2: attention ----------------
            with (
                tc.tile_pool(name="eposp", bufs=12) as eposp,
                tc.tile_pool(name="praw", bufs=3) as praw,
                tc.tile_pool(name="ptp", bufs=4) as ptp,
                tc.tile_pool(name="rcp", bufs=2) as rcp,
                tc.tile_pool(name="bcp", bufs=2) as bcp,
                tc.tile_pool(name="psS", bufs=2, space="PSUM") as psS,
                tc.tile_pool(name="psO", bufs=2, space="PSUM") as psO,
            ):
                LAG = 3

                def emit_S(h, k):
                    p, sub = divmod(h, 2)
                    s_ps = psS.tile([128, L], f32, tag="sps")
                    for hf in range(2):
                        nc.tensor.matmul(
                            s_ps[:, hf * 512:(hf + 1) * 512],
                            KT[sub * 64:(sub + 1) * 64, p, k * 128:(k + 1) * 128],
                            QT[sub * 64:(sub + 1) * 64, p, hf * 512:(hf + 1) * 512],
                        )
                    pr = praw.tile([128, L], f16, tag="pr")
                    nc.scalar.activation(pr[:], s_ps[:], AF.Exp, bias=expb[:])
                    ep = eposp.tile([128, L], f16, tag="ep")
                    nc.sync.dma_start(ep[:], epos[h, k * 128:(k + 1) * 128, :])
                    pt = ptp.tile([128, L], f16, tag="pt")
                    eng = nc.gpsimd if (k % 3 == 0) else nc.vector
                    eng.tensor_mul(pt[:], pr[:], ep[:])
                    return pt

                for h in range(H):
                    p, sub = divmod(h, 2)
                    pts = [emit_S(h, k) for k in range(LAG)]
                    o_ps = psO.tile([65, L], f32)
                    for k in range(KC):
                        if k + LAG < KC:
                            pts.append(emit_S(h, k + LAG))
                        for hf in range(2):
                            nc.tensor.matmul(
                                o_ps[:, hf * 512:(hf + 1) * 512],
                                Vt[k][:, h, :],
                                pts[k][:, hf * 512:(hf + 1) * 512],
                                start=(k == 0),
                                stop=(k == KC - 1),
                            )
                    # normalize straight out of PSUM: recip of the rowsum row,
                    # broadcast, then one vector multiply psum -> XT. Early
                    # heads use a DMA round-trip broadcast (off the compute
                    # engines); the last two heads gate phase 3, so they take
                    # the lower-latency Pool-engine broadcast instead.
                    rr = rcp.tile([1, L], f32, tag="rr")
                    nc.vector.tensor_copy(rr[:], o_ps[64:65, :])
                    rr2 = rcp.tile([1, L], f32, tag="rr2")
                    nc.vector.reciprocal_approx_fast(rr2[:], rr[:])
                    bc = bcp.tile([64, L], f32, tag="bc")
                    if h < H - 2:
                        nc.sync.dma_start(rscr[h:h + 1, :], rr2[:])
                        nc.sync.dma_start(
                            bc[:], rscr[h:h + 1, :].broadcast_to([64, L])
                        )
                    else:
                        nc.gpsimd.partition_broadcast(bc[:], rr2[:])
                    nc.vector.tensor_mul(
                        XT[sub * 64:(sub + 1) * 64, p, :], o_ps[0:64, :], bc[:]
                    )

            # ---------------- phase 3: output projection ----------------
            with (
                tc.tile_pool(name="outp", bufs=2) as outp,
                tc.tile_pool(name="psOut", bufs=2, space="PSUM") as psOut,
            ):
                # software-pipelined: each chunk's first 5 contraction steps
                # (heads 0-9) are emitted ahead of the previous chunk's last
                # step (d=5, heads 10/11), so the PE has ready work while the
                # final heads' normalize completes.
                def emit_head(qc, d):
                    for lo, sz in ((0, 512), (512, 256)):
                        nc.tensor.matmul(
                            ps_q[qc][:, lo:lo + sz],
                            XT[:, d, qc * 128:(qc + 1) * 128],
                            wp_sb[:, d, lo:lo + sz],
                            start=(d == 0),
                            stop=(d == CP - 1),
                        )

                def emit_tail(qc):
                    emit_head(qc, CP - 1)
                    ot = outp.tile([128, DIM], f32)
                    nc.vector.tensor_add(ot[:], ps_q[qc][:], bias_bc[:])
                    nc.sync.dma_start(out[qc * 128:(qc + 1) * 128, :], ot[:])

                ps_q = {}
                for qc in range(KC):
                    ps_q[qc] = psOut.tile([128, DIM], f32, name=f"psq{qc}",
                                          tag="psq")
                    for d in range(CP - 1):
                        emit_head(qc, d)
                    if qc >= 1:
                        emit_tail(qc - 1)
                emit_tail(KC - 1)

    nc.compile()
    return nc


def _get_nc():
    if "nc" not in _CACHE:
        _CACHE["nc"] = _build()
    return _CACHE["nc"]


def _host_prep(q, kv, attn_pos, Wq, Wkv, Wproj, bproj):
    import ml_dtypes

    bf16 = ml_dtypes.bfloat16

    q = np.asarray(q, dtype=np.float32)
    kv = np.asarray(kv, dtype=np.float32)
    attn_pos = np.asarray(attn_pos, dtype=np.float32)
    Wq = np.asarray(Wq, dtype=np.float32)
    Wkv = np.asarray(Wkv, dtype=np.float32)
    Wproj = np.asarray(Wproj, dtype=np.float32)
    bproj = np.asarray(bproj, dtype=np.float32)

    wq = np.ascontiguousarray((Wq * SCALE).T).astype(bf16)   # [c, d]
    wk = np.ascontiguousarray(Wkv[:DIM].T).astype(bf16)      # [c, d]
    wv = np.ascontiguousarray(Wkv[DIM:].T).astype(bf16)      # [c, d]
    wp = np.ascontiguousarray(Wproj.T).astype(bf16)          # [d, e]
    bias = np.ascontiguousarray(np.tile(bproj[None, :], (128, 1)))
    # epos[h, k, q] = exp(attn_pos[0, h, q, k])
    epos = np.ascontiguousarray(
        np.exp(attn_pos[0]).transpose(0, 2, 1)
    ).astype(np.float16)

    qT = np.ascontiguousarray(q.transpose(0, 2, 1)).astype(bf16)    # [B, c, L]
    kvT = np.ascontiguousarray(kv.transpose(0, 2, 1)).astype(bf16)  # [B, c, L]

    shared = {"wq": wq, "wk": wk, "wv": wv, "wp": wp, "bias": bias, "epos": epos}
    in_maps = []
    for b in range(B):
        m = dict(shared)
        m["qT"] = qT[b]
        m["kvT"] = kvT[b]
        in_maps.append(m)
    return in_maps


def kernel(q, kv, attn_pos, Wq, Wkv, Wproj, bproj):
    from concourse.bass_utils import run_bass_kernel_spmd

    nc = _get_nc()
    in_maps = _host_prep(q, kv, attn_pos, Wq, Wkv, Wproj, bproj)
    res = run_bass_kernel_spmd(nc, in_maps, list(range(NCORES)))
    return np.stack([res.results[b]["out"] for b in range(B)], axis=0)


# revision 27
# speedup vs baseline: 1.0489x; 1.0489x over previous
"""CrossAttention Trainium2 kernel.

Full inputs in, full output out. Data-parallel over batch: core b computes
batch item b of 8.

Per-core math (layouts chosen so the PE contraction dim is always the
partition dim, no on-chip transposes):
  QT[d, q] = (Wq*scale @ q_b^T)      lhsT=wq chunks, rhs=q_b^T    (bf16)
  KT[d, k] = (Wk @ kv_b^T)                                        (bf16)
  V[k, d]  = (kv_b @ Wv^T)           + ones column -> rowsum row 64
  S^T[k, q] = K Q^T per head         (64-contraction)
  P^T = exp(S^T - ln256) * exp_posT  (exp_pos host-precomputed fp16)
  O^T[d, q] rows 0..63 (+rowsum row 64): psum-accumulated over k chunks
  XT = O^T[0:64] * (1/rowsum)        rowsum row copied to partition 0
                                     (custom DVE recip needs base-0 input;
                                     64-wide accesses must be 32-aligned)
  out[q, e] = XT^T @ WprojT + bias

Schedule: attention S-matmuls run 3 k-chunks ahead of the O-matmuls so the
exp->mul chain (scalar + pool/vector engines) never stalls the PE; epos
multiplies alternate between the Pool and Vector engines.
"""

import numpy as np

B, L, DIM, H, HD = 8, 1024, 768, 12, 64
NCORES = 8
CP = DIM // 128  # 6 chunks of the contraction/feature dim
KC = L // 128    # 8 k-chunks
SCALE = HD ** -0.5
LN_OFF = float(np.log(256.0))

_CACHE = {}


def _build():
    import concourse.bass as bass
    import concourse.mybir as mybir
    import concourse.tile as tile
    from concourse import bacc

    f32 = mybir.dt.float32
    bf16 = mybir.dt.bfloat16
    f16 = mybir.dt.float16
    f8 = mybir.dt.float8e4
    DR = mybir.MatmulPerfMode.DoubleRow
    AF = mybir.ActivationFunctionType

    nc = bacc.Bacc("TRN2", target_bir_lowering=False, debug=False)

    qT = nc.dram_tensor("qT", [DIM, L], bf16, kind="ExternalInput")
    kvT = nc.dram_tensor("kvT", [DIM, L], bf16, kind="ExternalInput")
    wq = nc.dram_tensor("wq", [DIM, DIM], bf16, kind="ExternalInput")  # [c, d]
    wk = nc.dram_tensor("wk", [DIM, DIM], bf16, kind="ExternalInput")  # [c, d]
    wv = nc.dram_tensor("wv", [DIM, DIM], bf16, kind="ExternalInput")  # [c, d]
    wp = nc.dram_tensor("wp", [DIM, DIM], bf16, kind="ExternalInput")  # [d, e]
    bias = nc.dram_tensor("bias", [128, DIM], f32, kind="ExternalInput")
    epos = nc.dram_tensor("epos", [H, L, L], f16, kind="ExternalInput")  # [h,k,q]
    out = nc.dram_tensor("out", [L, DIM], f32, kind="ExternalOutput")
    rscr = nc.dram_tensor("rs_scratch", [H, L], f32)

    with tile.TileContext(nc) as tc:
        with tc.tile_pool(name="persist", bufs=1) as persist:
            QT = persist.tile([128, CP, L], bf16)   # pair p: heads 2p, 2p+1
            KT = persist.tile([128, CP, L], bf16)
            Vt = [
                persist.tile([128, H, HD + 1], f16, name=f"Vt{k}", tag=f"V{k}")
                for k in range(KC)
            ]
            wp_sb = persist.tile([128, CP, DIM], bf16)
            bias_bc = persist.tile([128, DIM], f32)
            XT = persist.tile([128, CP, L], bf16)
            expb = persist.tile([128, 1], f32)
            nc.vector.memset(expb[:], -LN_OFF)

            # ---------------- phase 1: projections ----------------
            with (
                tc.tile_pool(name="ph1", bufs=1) as ph1,
                tc.tile_pool(name="psA", bufs=2, space="PSUM") as psA,
            ):
                q_sb = ph1.tile([128, CP, L], bf16)
                kv_sb = ph1.tile([128, CP, L], bf16)
                wq_sb = ph1.tile([128, CP, DIM], bf16)
                wk_sb = ph1.tile([128, CP, DIM], bf16)
                wv_sb = ph1.tile([128, CP, DIM], bf16)
                # issue DMAs in need-order: Q-proj inputs first
                for c in range(CP):
                    nc.sync.dma_start(wq_sb[:, c, :], wq[c * 128:(c + 1) * 128, :])
                    nc.sync.dma_start(q_sb[:, c, :], qT[c * 128:(c + 1) * 128, :])
                for c in range(CP):
                    nc.sync.dma_start(wk_sb[:, c, :], wk[c * 128:(c + 1) * 128, :])
                    nc.sync.dma_start(kv_sb[:, c, :], kvT[c * 128:(c + 1) * 128, :])
                for c in range(CP):
                    nc.sync.dma_start(wv_sb[:, c, :], wv[c * 128:(c + 1) * 128, :])
                for d in range(CP):
                    nc.sync.dma_start(wp_sb[:, d, :], wp[d * 128:(d + 1) * 128, :])
                nc.sync.dma_start(bias_bc[:], bias[:])

                for w_sb, x_sb, dst in ((wq_sb, q_sb, QT), (wk_sb, kv_sb, KT)):
                    for p in range(CP):
                        ps = psA.tile([128, L], f32, tag="proj")
                        for c in range(CP):
                            for hf in range(2):
                                nc.tensor.matmul(
                                    ps[:, hf * 512:(hf + 1) * 512],
                                    w_sb[:, c, p * 128:(p + 1) * 128],
                                    x_sb[:, c, hf * 512:(hf + 1) * 512],
                                    start=(c == 0),
                                    stop=(c == CP - 1),
                                )
                        nc.vector.tensor_copy(dst[:, p, :], ps[:])

                for k in range(KC):
                    ps = psA.tile([128, DIM], f32, tag="proj")
                    for c in range(CP):
                        for lo, sz in ((0, 512), (512, 256)):
                            nc.tensor.matmul(
                                ps[:, lo:lo + sz],
                                kv_sb[:, c, k * 128:(k + 1) * 128],
                                wv_sb[:, c, lo:lo + sz],
                                start=(c == 0),
                                stop=(c == CP - 1),
                            )
                    nc.vector.memset(Vt[k][:, :, HD:HD + 1], 1.0)
                    nc.vector.tensor_copy(
                        Vt[k][:, :, 0:HD],
                        ps[:].rearrange("p (h d) -> p h d", d=HD),
                    )

            # ---------------- phase 2: attention ----------------
            with (
                tc.tile_pool(name="eposp", bufs=12) as eposp,
                tc.tile_pool(name="praw", bufs=3) as praw,
                tc.tile_pool(name="ptp", bufs=4) as ptp,
                tc.tile_pool(name="rcp", bufs=2) as rcp,
                tc.tile_pool(name="bcp", bufs=2) as bcp,
                tc.tile_pool(name="psS", bufs=2, space="PSUM") as psS,
                tc.tile_pool(name="psO", bufs=2, space="PSUM") as psO,
            ):
                LAG = 3

                def emit_S(h, k):
                    p, sub = divmod(h, 2)
                    s_ps = psS.tile([128, L], f32, tag="sps")
                    for hf in range(2):
                        nc.tensor.matmul(
                            s_ps[:, hf * 512:(hf + 1) * 512],
                            KT[sub * 64:(sub + 1) * 64, p, k * 128:(k + 1) * 128],
                            QT[sub * 64:(sub + 1) * 64, p, hf * 512:(hf + 1) * 512],
                        )
                    pr = praw.tile([128, L], f16, tag="pr")
                    nc.scalar.activation(pr[:], s_ps[:], AF.Exp, bias=expb[:])
                    ep = eposp.tile([128, L], f16, tag="ep")
                    nc.sync.dma_start(ep[:], epos[h, k * 128:(k + 1) * 128, :])
                    pt = ptp.tile([128, L], f16, tag="pt")
                    eng = nc.gpsimd if (k % 3 == 0) else nc.vector
                    eng.tensor_mul(pt[:], pr[:], ep[:])
                    return pt

                for h in range(H):
                    p, sub = divmod(h, 2)
                    pts = [emit_S(h, k) for k in range(LAG)]
                    o_ps = psO.tile([65, L], f32)
                    for k in range(KC):
                        if k + LAG < KC:
                            pts.append(emit_S(h, k + LAG))
                        for hf in range(2):
                            nc.tensor.matmul(
                                o_ps[:, hf * 512:(hf + 1) * 512],
                                Vt[k][:, h, :],
                                pts[k][:, hf * 512:(hf + 1) * 512],
                                start=(k == 0),
                                stop=(k == KC - 1),
                            )
                    # normalize straight out of PSUM: recip of the rowsum row,
                    # broadcast, then one vector multiply psum -> XT. Early
                    # heads use a DMA round-trip broadcast (off the compute
                    # engines); the last two heads gate phase 3, so they take
                    # the lower-latency Pool-engine broadcast instead.
                    rr = rcp.tile([1, L], f32, tag="rr")
                    nc.vector.tensor_copy(rr[:], o_ps[64:65, :])
                    rr2 = rcp.tile([1, L], f32, tag="rr2")
                    nc.vector.reciprocal_approx_fast(rr2[:], rr[:])
                    bc = bcp.tile([64, L], f32, tag="bc")
                    nc.sync.dma_start(rscr[h:h + 1, :], rr2[:])
                    nc.sync.dma_start(
                        bc[:], rscr[h:h + 1, :].broadcast_to([64, L])
                    )
                    nc.vector.tensor_mul(
                        XT[sub * 64:(sub + 1) * 64, p, :], o_ps[0:64, :], bc[:]
                    )

            # ---------------- phase 3: output projection ----------------
            with (
                tc.tile_pool(name="outp", bufs=2) as outp,
                tc.tile_pool(name="psOut", bufs=2, space="PSUM") as psOut,
            ):
                # software-pipelined: each chunk's first 5 contraction steps
                # (heads 0-9) are emitted ahead of the previous chunk's last
                # step (d=5, heads 10/11), so the PE has ready work while the
                # final heads' normalize completes.
                def emit_head(qc, d):
                    for lo, sz in ((0, 512), (512, 256)):
                        nc.tensor.matmul(
                            ps_q[qc][:, lo:lo + sz],
                            XT[:, d, qc * 128:(qc + 1) * 128],
                            wp_sb[:, d, lo:lo + sz],
                            start=(d == 0),
                            stop=(d == CP - 1),
                        )

                def emit_tail(qc):
                    emit_head(qc, CP - 1)
                    ot = outp.tile([128, DIM], f32)
                    nc.vector.tensor_add(ot[:], ps_q[qc][:], bias_bc[:])
                    nc.sync.dma_start(out[qc * 128:(qc + 1) * 128, :], ot[:])

                ps_q = {}
                for qc in range(KC):
                    ps_q[qc] = psOut.tile([128, DIM], f32, name=f"psq{qc}",
                                          tag="psq")
                    for d in range(CP - 1):
                        emit_head(qc, d)
                    if qc >= 1:
                        emit_tail(qc - 1)
                emit_tail(KC - 1)

    nc.compile()
    return nc


def _get_nc():
    if "nc" not in _CACHE:
        _CACHE["nc"] = _build()
    return _CACHE["nc"]


def _host_prep(q, kv, attn_pos, Wq, Wkv, Wproj, bproj):
    import ml_dtypes

    bf16 = ml_dtypes.bfloat16

    q = np.asarray(q, dtype=np.float32)
    kv = np.asarray(kv, dtype=np.float32)
    attn_pos = np.asarray(attn_pos, dtype=np.float32)
    Wq = np.asarray(Wq, dtype=np.float32)
    Wkv = np.asarray(Wkv, dtype=np.float32)
    Wproj = np.asarray(Wproj, dtype=np.float32)
    bproj = np.asarray(bproj, dtype=np.float32)

    wq = np.ascontiguousarray((Wq * SCALE).T).astype(bf16)   # [c, d]
    wk = np.ascontiguousarray(Wkv[:DIM].T).astype(bf16)      # [c, d]
    wv = np.ascontiguousarray(Wkv[DIM:].T).astype(bf16)      # [c, d]
    wp = np.ascontiguousarray(Wproj.T).astype(bf16)          # [d, e]
    bias = np.ascontiguousarray(np.tile(bproj[None, :], (128, 1)))
    # epos[h, k, q] = exp(attn_pos[0, h, q, k])
    epos = np.ascontiguousarray(
        np.exp(attn_pos[0]).transpose(0, 2, 1)
    ).astype(np.float16)

    qT = np.ascontiguousarray(q.transpose(0, 2, 1)).astype(bf16)    # [B, c, L]
    kvT = np.ascontiguousarray(kv.transpose(0, 2, 1)).astype(bf16)  # [B, c, L]

    shared = {"wq": wq, "wk": wk, "wv": wv, "wp": wp, "bias": bias, "epos": epos}
    in_maps = []
    for b in range(B):
        m = dict(shared)
        m["qT"] = qT[b]
        m["kvT"] = kvT[b]
        in_maps.append(m)
    return in_maps


def kernel(q, kv, attn_pos, Wq, Wkv, Wproj, bproj):
    from concourse.bass_utils import run_bass_kernel_spmd

    nc = _get_nc()
    in_maps = _host_prep(q, kv, attn_pos, Wq, Wkv, Wproj, bproj)
    res = run_bass_kernel_spmd(nc, in_maps, list(range(NCORES)))
    return np.stack([res.results[b]["out"] for b in range(B)], axis=0)
